# Initial kernel scaffold
#
"""Causal single-head attention (B=4, T=4096, C=1024, D=64) on 8 NeuronCores.

Sharding: core c = (batch b = c % 4, half h = c // 4).
Each core handles ALL queries of its batch, but only its half of the key
blocks (256-token key blocks with block index ≡ h mod 2).  This makes the
program identical on every core (pure SPMD, no control flow); cores differ
only in input data.  Each core emits unnormalized partial results
U^T = [V|1]^T @ exp(S^T) per query supertile; the host combines the two
halves per batch: O = (U0 + U1)[:64] / (U0 + U1)[64].

On-chip dataflow (all bf16 except PSUM/f32 accumulators):
  xq^T [C, T]   -> Q^T [64, T]          (matmul, C-tiled accumulation)
  xk^T [C, T/2] -> K^T, V^T [65, T/2]   (V^T row 64 = ones, for row-sums)
  V' [128, 65] per key tile              (PE transpose of V^T)
  S^T [128k, 512q] = K_tile @ Q^T        (matmul, contraction over D=64)
  P = exp(S^T/8) * causal_mask           (ACT exp from PSUM, DVE mask mul)
  U^T [65, 512] += V'_j^T @ P_j          (matmul, contraction over 128 keys)
"""
import sys
import numpy as np
import ml_dtypes

if "/opt/trn_rl_repo" not in sys.path:
    sys.path.insert(0, "/opt/trn_rl_repo")

import concourse.bacc as bacc
import concourse.mybir as mybir
from concourse import tile
from concourse import bass_utils

bf16 = mybir.dt.bfloat16
f32 = mybir.dt.float32
BF = ml_dtypes.bfloat16

B, T, C, D = 4, 4096, 1024, 64
NST = 8          # query supertiles per batch (512 queries each)
STQ = 512
TK = T // 2      # key tokens per core
NKT = TK // 128  # local 128-key tiles per core (16)
NC_ = C // 128   # 8 c-tiles

_CACHE = {}


def _build():
    nc = bacc.Bacc(None, target_bir_lowering=False, debug=False, num_devices=8)

    xq = nc.dram_tensor("xq", [C, T], bf16, kind="ExternalInput")
    xk = nc.dram_tensor("xk", [C, TK], bf16, kind="ExternalInput")
    w = nc.dram_tensor("w", [C, 192], bf16, kind="ExternalInput")   # Wq|Wk|Wv
    msk = nc.dram_tensor("msk", [256, STQ], bf16, kind="ExternalInput")
    idn = nc.dram_tensor("idn", [65, 65], bf16, kind="ExternalInput")
    out = nc.dram_tensor("out", [65, T], f32, kind="ExternalOutput")

    with tile.TileContext(nc) as tc:
        with tc.tile_pool(name="sb", bufs=1) as sb, \
             tc.tile_pool(name="pp", bufs=3) as pp, \
             tc.tile_pool(name="ps", bufs=2, space="PSUM") as ps:

            # ---- resident inputs ----
            xq_t = [sb.tile([128, T], bf16, tag=f"xq{c}", name=f"xq{c}")
                    for c in range(NC_)]
            xk_t = [sb.tile([128, TK], bf16, tag=f"xk{c}", name=f"xk{c}")
                    for c in range(NC_)]
            w_t = [sb.tile([128, 192], bf16, tag=f"w{c}", name=f"w{c}")
                   for c in range(NC_)]
            msk_t = sb.tile([128, 2 * STQ], bf16, tag="msk")
            idn_t = sb.tile([65, 65], bf16, tag="idn")
            for c in range(NC_):
                nc.sync.dma_start(xq_t[c][:], xq[128 * c:128 * (c + 1), :])
                nc.sync.dma_start(xk_t[c][:], xk[128 * c:128 * (c + 1), :])
                nc.sync.dma_start(w_t[c][:], w[128 * c:128 * (c + 1), :])
            nc.sync.dma_start(msk_t[:, 0:STQ], msk[0:128, :])
            nc.sync.dma_start(msk_t[:, STQ:2 * STQ], msk[128:256, :])
            nc.sync.dma_start(idn_t[:], idn[:])

            # ---- persistent intermediates ----
            qT = sb.tile([64, T], bf16, tag="qT")
            kT = sb.tile([64, TK], bf16, tag="kT")
            vT = sb.tile([65, TK], bf16, tag="vT")   # row 64 = ones
            vP = sb.tile([128, NKT * 65], bf16, tag="vP")  # V' tiles

            nc.vector.memset(vT[64:65, :], 1.0)

            # ---- projections ----
            # Q^T over all tokens, 512-wide chunks
            for st in range(NST):
                acc = ps.tile([64, STQ], f32, tag="work")
                for c in range(NC_):
                    nc.tensor.matmul(acc[:], w_t[c][:, 0:64],
                                     xq_t[c][:, STQ * st:STQ * (st + 1)],
                                     start=(c == 0), stop=(c == NC_ - 1))
                nc.vector.tensor_copy(qT[:, STQ * st:STQ * (st + 1)], acc[:])

            # K^T / V^T over local key tokens, 256-wide chunks
            for blk in range(TK // 256):
                sl = slice(256 * blk, 256 * (blk + 1))
                acck = ps.tile([64, 256], f32, tag="work")
                for c in range(NC_):
                    nc.tensor.matmul(acck[:], w_t[c][:, 64:128], xk_t[c][:, sl],
                                     start=(c == 0), stop=(c == NC_ - 1))
                nc.vector.tensor_copy(kT[:, sl], acck[:])
                accv = ps.tile([64, 256], f32, tag="work")
                for c in range(NC_):
                    nc.tensor.matmul(accv[:], w_t[c][:, 128:192], xk_t[c][:, sl],
                                     start=(c == 0), stop=(c == NC_ - 1))
                nc.vector.tensor_copy(vT[0:64, sl], accv[:])

            # V' tiles: transpose V^T (incl. ones row) per 128-key tile
            for j in range(NKT):
                tp = ps.tile([128, 65], bf16, tag="work")
                nc.tensor.transpose(tp[:], vT[:, 128 * j:128 * (j + 1)], idn_t[:])
                nc.vector.tensor_copy(vP[:, 65 * j:65 * (j + 1)], tp[:])

            # ---- attention ----
            for st in range(NST):
                qsl = slice(STQ * st, STQ * (st + 1))
                n = 2 * (st + 1)          # local key tiles for this supertile
                u = ps.tile([65, STQ], f32, tag="u")
                for j0 in range(0, n, 2):
                    s2 = ps.tile([128, 2 * STQ], f32, tag="s")
                    p2 = pp.tile([128, 2 * STQ], bf16, tag="p")
                    for d in range(2):
                        j = j0 + d
                        nc.tensor.matmul(s2[:, STQ * d:STQ * (d + 1)],
                                         kT[:, 128 * j:128 * (j + 1)],
                                         qT[:, qsl], start=True, stop=True)
                    nc.scalar.activation(p2[:], s2[:],
                                         mybir.ActivationFunctionType.Exp,
                                         scale=0.125)
                    if j0 == n - 2:  # diagonal pair -> causal masks
                        nc.vector.tensor_mul(p2[:], p2[:], msk_t[:])
                    for d in range(2):
                        j = j0 + d
                        nc.tensor.matmul(u[:], vP[:, 65 * j:65 * (j + 1)],
                                         p2[:, STQ * d:STQ * (d + 1)],
                                         start=(j == 0), stop=(j == n - 1))
                u_sb = pp.tile([65, STQ], f32, tag="u_sb")
                nc.vector.tensor_copy(u_sb[:], u[:])
                nc.sync.dma_start(out[:, qsl], u_sb[:])

    nc.compile()
    return nc


def _get_nc():
    if "nc" not in _CACHE:
        _CACHE["nc"] = _build()
    return _CACHE["nc"]


def kernel(x, Wq, Wk, Wv, _trace=False):
    x = np.asarray(x)
    nc = _get_nc()

    xT = np.ascontiguousarray(x.transpose(0, 2, 1)).astype(BF)   # [B, C, T]
    w = np.concatenate([Wq, Wk, Wv], axis=1).astype(BF)          # [C, 192]
    idn = np.eye(65, dtype=BF)

    j = np.arange(128)[:, None]
    i = np.arange(STQ)[None, :]
    masks = {}
    for h in range(2):
        m0 = (j <= i - 256 * h).astype(BF)
        m1 = (j <= i - 256 * h - 128).astype(BF)
        masks[h] = np.concatenate([m0, m1], axis=0)

    # key-token selector: 256-blocks with block index ≡ h (mod 2)
    tok = np.arange(T)
    keysel = {h: ((tok // 256) % 2 == h) for h in range(2)}

    in_maps = []
    for c in range(8):
        b, h = c % 4, c // 4
        in_maps.append({
            "xq": xT[b],
            "xk": np.ascontiguousarray(xT[b][:, keysel[h]]),
            "w": w,
            "msk": masks[h],
            "idn": idn,
        })

    res = bass_utils.run_bass_kernel_spmd(nc, in_maps, core_ids=list(range(8)),
                                          trace=_trace)
    _CACHE["last_results"] = res

    O = np.empty((B, T, D), dtype=np.float32)
    for b in range(B):
        U = res.results[b]["out"] + res.results[b + 4]["out"]    # [65, T]
        O[b] = (U[:D] / U[D:D + 1]).T
    return O



# revision 7
# speedup vs baseline: 1.2931x; 1.2931x over previous
"""Causal single-head attention (B=4, T=4096, C=1024, D=64) on 8 NeuronCores.

Sharding: core c = (batch b = c % 4, half h = c // 4).
Each core handles ALL queries of its batch, but only its half of the key
blocks (256-token key blocks with block index ≡ h mod 2).  Identical program
on every core (pure SPMD); cores differ only in input data (xk gather + mask).
Each core emits unnormalized partial results U^T = [V|1]^T @ exp(S^T) per
query supertile; the host combines: O = (U0 + U1)[:64] / (U0 + U1)[64] / 8.

v2 dataflow (fp8 inputs, DoubleRow projections/PV, row-tiled scores):
  x^T, W in fp8e4 (W pre-scaled x8); contraction pairs packed [128, 2, *]
  Q^T  [64, T]  = DoubleRow matmul (Wq stationary), copied to BOTH halves of
                  qT2 [128, T] bf16 (Pool duplicates rows 64:128)
  K^T/V^T       = DoubleRow matmul ([Wk|Wv] stationary, M=128): K rows 0:64,
                  V rows 64:128 of PSUM; K tiles -> kT2 [128, 8*128] bf16
                  (even local tile in rows 0:64, odd in 64:128), V -> vT
                  [65, TK] bf16 (row 64 = ones)
  V' tiles      = PE transpose of vT per 128-key tile -> vP [128, 16, 65] fp8
  S^T pair      = TWO concurrent row-tiled matmuls (tile_position (0,0) and
                  (64,0), contraction D=64 each) -> s2 [128, 2, 512] f32 PSUM
  P             = ACT exp(s2/512 - 2.5) -> fp8 (range-safe; constant factor
                  cancels in normalization); diagonal pair masked on DVE
  U^T [65, 512] += DoubleRow matmul (vP pair stationary [128,2,65], p2 moving
                  [128,2,512], contraction 256 = two key tiles per matmul)
"""
import sys
import numpy as np
import ml_dtypes

if "/opt/trn_rl_repo" not in sys.path:
    sys.path.insert(0, "/opt/trn_rl_repo")

import concourse.bacc as bacc
import concourse.mybir as mybir
from concourse import tile
from concourse import bass_utils

bf16 = mybir.dt.bfloat16
f32 = mybir.dt.float32
f8 = mybir.dt.float8e4
BF = ml_dtypes.bfloat16
F8 = ml_dtypes.float8_e4m3
DR = mybir.MatmulPerfMode.DoubleRow

B, T, C, D = 4, 4096, 1024, 64
NST = 8          # query supertiles per batch (512 queries each)
STQ = 512
TK = T // 2      # key tokens per core
NKT = TK // 128  # local 128-key tiles per core (16)
NPT = C // 256   # contraction pair-tiles (4)
WS = 8.0         # host-side weight scale (fp8 range)

_CACHE = {}


def _build():
    nc = bacc.Bacc(None, target_bir_lowering=False, debug=False, num_devices=8)

    xq = nc.dram_tensor("xq", [128, 2 * NPT, T], f8, kind="ExternalInput")
    xk = nc.dram_tensor("xk", [128, 2 * NPT, TK], f8, kind="ExternalInput")
    wq = nc.dram_tensor("wq", [128, 2 * NPT, 64], f8, kind="ExternalInput")
    wkv = nc.dram_tensor("wkv", [128, 2 * NPT, 128], f8, kind="ExternalInput")
    msk = nc.dram_tensor("msk", [128, 2, STQ], f8, kind="ExternalInput")
    idn = nc.dram_tensor("idn", [65, 65], bf16, kind="ExternalInput")
    out = nc.dram_tensor("out", [65, T], f32, kind="ExternalOutput")

    with tile.TileContext(nc) as tc:
        with tc.tile_pool(name="sb", bufs=1) as sb, \
             tc.tile_pool(name="pp", bufs=3) as pp, \
             tc.tile_pool(name="ps", bufs=2, space="PSUM") as ps:

            # ---- resident inputs ----
            xq_sb = sb.tile([128, 2 * NPT, T], f8, tag="xq")
            xk_sb = sb.tile([128, 2 * NPT, TK], f8, tag="xk")
            wq_sb = sb.tile([128, 2 * NPT, 64], f8, tag="wq")
            wkv_sb = sb.tile([128, 2 * NPT, 128], f8, tag="wkv")
            msk_sb = sb.tile([128, 2, STQ], f8, tag="msk")
            idn_sb = sb.tile([65, 65], bf16, tag="idn")

            nc.sync.dma_start(wq_sb[:], wq[:])
            nc.sync.dma_start(wkv_sb[:], wkv[:])
            nc.sync.dma_start(msk_sb[:], msk[:])
            nc.sync.dma_start(idn_sb[:], idn[:])
            # interleave xk (feeds K/V, needed first) with xq chunks
            for c in range(4):
                nc.sync.dma_start(xk_sb[:, :, 512 * c:512 * (c + 1)],
                                  xk[:, :, 512 * c:512 * (c + 1)])
                for q in (2 * c, 2 * c + 1):
                    nc.sync.dma_start(xq_sb[:, :, 512 * q:512 * (q + 1)],
                                      xq[:, :, 512 * q:512 * (q + 1)])

            # ---- persistent intermediates ----
            qT2 = sb.tile([128, T], bf16, tag="qT2")   # rows 64:128 duplicate
            kT2 = sb.tile([128, NKT // 2, 128], bf16, tag="kT2")
            vT = sb.tile([65, TK], bf16, tag="vT")     # row 64 = ones
            # inner dim padded 65->80: DoubleRow weights need ko-stride %16==0
            vP = sb.tile([128, NKT, 80], f8, tag="vP")

            ebias = sb.tile([128, 1], f32, tag="ebias")
            nc.vector.memset(ebias[:], -2.5)
            nc.vector.memset(vT[64:65, :], 1.0)

            def kv_chunk(c):
                sl = slice(512 * c, 512 * (c + 1))
                acc = ps.tile([128, 512], f32, tag="work", name=f"kv{c}")
                for t in range(NPT):
                    nc.tensor.matmul(acc[:], wkv_sb[:, 2 * t:2 * t + 2, :],
                                     xk_sb[:, 2 * t:2 * t + 2, sl],
                                     start=(t == 0), stop=(t == NPT - 1),
                                     perf_mode=DR)
                for i in range(4):
                    tau = 4 * c + i            # local key tile
                    pj, po = tau // 2, tau % 2
                    nc.vector.tensor_copy(
                        kT2[64 * po:64 * po + 64, pj, :],
                        acc[0:64, 128 * i:128 * (i + 1)])
                nc.vector.tensor_copy(vT[0:64, sl], acc[64:128, :])
                for i in range(4):
                    tau = 4 * c + i
                    tp = ps.tile([128, 65], bf16, tag="work", name=f"tp{tau}")
                    nc.tensor.transpose(tp[:], vT[:, 128 * tau:128 * (tau + 1)],
                                        idn_sb[:])
                    nc.vector.tensor_copy(vP[:, tau, 0:65], tp[:])

            def q_chunk(q):
                sl = slice(512 * q, 512 * (q + 1))
                acc = ps.tile([64, 512], f32, tag="work", name=f"q{q}")
                for t in range(NPT):
                    nc.tensor.matmul(acc[:], wq_sb[:, 2 * t:2 * t + 2, :],
                                     xq_sb[:, 2 * t:2 * t + 2, sl],
                                     start=(t == 0), stop=(t == NPT - 1),
                                     perf_mode=DR)
                nc.vector.tensor_copy(qT2[0:64, sl], acc[:])
                nc.gpsimd.tensor_copy(qT2[64:128, sl], qT2[0:64, sl])

            def att(st):
                qsl = slice(STQ * st, STQ * (st + 1))
                npair = st + 1
                u = ps.tile([65, STQ], f32, tag="u", name=f"u{st}")
                for pj in range(npair):
                    s2 = ps.tile([128, 2, STQ], f32, tag="s", name=f"s{st}_{pj}")
                    p2 = pp.tile([128, 2, STQ], f8, tag="p", name=f"p{st}_{pj}")
                    nc.tensor.matmul(s2[:, 0, :], kT2[0:64, pj, :],
                                     qT2[0:64, qsl], start=True, stop=True)
                    nc.tensor.matmul(s2[:, 1, :], kT2[64:128, pj, :],
                                     qT2[64:128, qsl], start=True, stop=True)
                    nc.scalar.activation(p2[:], s2[:],
                                         mybir.ActivationFunctionType.Exp,
                                         scale=1.0 / (8.0 * WS * WS),
                                         bias=ebias[:])
                    if pj == npair - 1:     # diagonal pair -> causal masks
                        nc.vector.tensor_mul(p2[:], p2[:], msk_sb[:])
                    nc.tensor.matmul(u[:], vP[:, 2 * pj:2 * pj + 2, 0:65],
                                     p2[:], start=(pj == 0),
                                     stop=(pj == npair - 1), perf_mode=DR)
                u_sb = pp.tile([65, STQ], f32, tag="u_sb", name=f"us{st}")
                nc.vector.tensor_copy(u_sb[:], u[:])
                nc.sync.dma_start(out[:, qsl], u_sb[:])

            # ---- interleaved schedule: attention starts as soon as fed ----
            kv_chunk(0)
            q_chunk(0)
            att(0)
            q_chunk(1)
            att(1)
            kv_chunk(1)
            q_chunk(2)
            att(2)
            q_chunk(3)
            att(3)
            kv_chunk(2)
            q_chunk(4)
            att(4)
            q_chunk(5)
            att(5)
            kv_chunk(3)
            q_chunk(6)
            att(6)
            q_chunk(7)
            att(7)

    nc.compile()
    return nc


def _get_nc():
    if "nc" not in _CACHE:
        _CACHE["nc"] = _build()
    return _CACHE["nc"]


def _to_f8(a):
    return np.clip(a, -240.0, 240.0).astype(F8)


def _pack_pairs(a):
    """[C, N] -> [128, 2*NPT, N] with channel c = 256*t + 128*ko + p."""
    n = a.shape[1]
    return np.ascontiguousarray(
        a.reshape(NPT, 2, 128, n).transpose(2, 0, 1, 3).reshape(128, 2 * NPT, n))


def kernel(x, Wq, Wk, Wv, _trace=False):
    x = np.asarray(x)
    nc = _get_nc()

    wq_h = _to_f8(_pack_pairs(np.asarray(Wq) * WS))
    wkv_h = _to_f8(_pack_pairs(
        np.concatenate([np.asarray(Wk), np.asarray(Wv)], axis=1) * WS))
    idn = np.eye(65, dtype=BF)

    j = np.arange(128)[:, None]
    i = np.arange(STQ)[None, :]
    masks = {}
    for h in range(2):
        m0 = (j <= i - 256 * h).astype(F8)
        m1 = (j <= i - 256 * h - 128).astype(F8)
        masks[h] = np.stack([m0, m1], axis=1)          # [128, 2, 512]

    tok = np.arange(T)
    keysel = {h: ((tok // 256) % 2 == h) for h in range(2)}

    xq_h = []
    for b in range(B):
        xT = np.ascontiguousarray(x[b].T)              # [C, T]
        xq_h.append(_pack_pairs(_to_f8(xT).view(np.uint8)).view(F8))

    in_maps = []
    for c in range(8):
        b, h = c % 4, c // 4
        in_maps.append({
            "xq": xq_h[b],
            "xk": np.ascontiguousarray(xq_h[b][:, :, keysel[h]]),
            "wq": wq_h,
            "wkv": wkv_h,
            "msk": masks[h],
            "idn": idn,
        })

    res = bass_utils.run_bass_kernel_spmd(nc, in_maps, core_ids=list(range(8)),
                                          trace=_trace)
    _CACHE["last_results"] = res

    O = np.empty((B, T, D), dtype=np.float32)
    for b in range(B):
        U = res.results[b]["out"] + res.results[b + 4]["out"]    # [65, T]
        O[b] = (U[:D] / U[D:D + 1]).T / WS
    return O
